# revision 16
# baseline (speedup 1.0000x reference)
"""Bahdanau-attention kernel for Trainium2 (8 NeuronCores, batch-sharded).

The reference computes

    score  = tanh(features @ W1 + b1 + hidden @ W2 + b2) @ V + bv   # [B, 1, 1]
    attn   = softmax(score, axis=1)                                 # axis of size 1!
    context = sum(attn * features[:, None, :], axis=1)              # [B, D]

The softmax is taken over an axis of size 1, so `attn == 1.0` exactly and
`context == features` bitwise — the two matmuls and the tanh are dead code.
The roofline for this module is therefore pure memory traffic: read the
features shard and write it back as `context`, plus a tiny ones tensor.

Each core owns B/8 = 2048 rows: DRAM->DRAM DMA copy of its [2048, 2048] f32
features shard into the context output, and a memset(1.0) tile stored to the
attention-weights output.

Measured on trn2 (amplified-slope method, all 8 cores concurrent): 99-101 us
per core for the copy = ~333 GB/s of combined HBM traffic. One-directional
calibration gives 383 GB/s read-only and 381 GB/s write-only, but combined
read+write caps at ~333 GB/s no matter the structure: chunk count (1..16),
ring choice (SP/ACT HWDGE), adding the SWDGE queue family, and a
double-buffered HBM->SBUF->HBM pipeline with reads and writes on separate
rings all time identical. The copy is half-duplex-HBM-bound, so this is the
floor for device-materialized outputs. Cost model (TimelineSim) reports
50.2 us because it charges DRAM->DRAM at ~360 GB/s once, not per direction.
"""

import os

import numpy as np

import concourse.bass as bass
from concourse import mybir
from concourse.bass_utils import run_bass_kernel_spmd

N_CORES = 8
B, D, H = 16384, 2048, 1024
ROWS = B // N_CORES  # 2048 rows per core
N_SYNC_CHUNKS = 4  # copy chunks issued from the sync (SP) HWDGE ring
N_SCALAR_CHUNKS = 4  # copy chunks issued from the scalar (ACT) HWDGE ring

LAST_EXEC_TIME_NS = None
LAST_RESULTS = None

_NC = None


def _get_nc():
    global _NC
    if _NC is None:
        _NC = _build_nc()
    return _NC


def _build_nc():
    nc = bass.Bass(trn_type="TRN2")
    x = nc.dram_tensor("x", [ROWS, D], mybir.dt.float32, kind="ExternalInput")
    ctx_out = nc.dram_tensor("ctx", [ROWS, D], mybir.dt.float32, kind="ExternalOutput")
    attn_out = nc.dram_tensor(
        "attn", [128, ROWS // 128], mybir.dt.float32, kind="ExternalOutput"
    )

    total = N_SYNC_CHUNKS + N_SCALAR_CHUNKS
    rows_per = ROWS // total
    chunks = [slice(i * rows_per, (i + 1) * rows_per) for i in range(total)]
    sync_chunks = chunks[:N_SYNC_CHUNKS]
    scalar_chunks = chunks[N_SYNC_CHUNKS:]

    with (
        nc.sbuf_tensor([128, ROWS // 128], mybir.dt.float32) as ones,
        nc.semaphore("vsem") as vsem,
        nc.semaphore("attn_sem") as attn_sem,
        nc.semaphore("dma_sem") as dma_sem,
        nc.Block() as block,
    ):
        # The ones/attn path lives entirely on gpsimd (SWDGE, own semaphore),
        # keeping both HWDGE rings free to stream the big copy from t=0.
        @block.gpsimd
        def _(gpsimd):
            gpsimd.memset(ones[:], 1.0).then_inc(vsem, 1)
            gpsimd.wait_ge(vsem, 1)
            gpsimd.dma_start(out=attn_out[:, :], in_=ones[:]).then_inc(attn_sem, 16)

        @block.scalar
        def _(scalar):
            for sl in scalar_chunks:
                scalar.dma_start(out=ctx_out[sl, :], in_=x[sl, :]).then_inc(
                    dma_sem, 16
                )

        @block.sync
        def _(sync):
            for sl in sync_chunks:
                sync.dma_start(out=ctx_out[sl, :], in_=x[sl, :]).then_inc(dma_sem, 16)
            sync.wait_ge(dma_sem, 16 * total)
            sync.wait_ge(attn_sem, 16)

    return nc


def _warm_compile():
    """Best-effort: trigger the neuronx-cc NEFF compile at import time via a
    lower().compile() of the same jitted body run_bass_kernel_spmd will build,
    so the first kernel() call hits the on-disk NEFF cache instead of paying
    the compile. No data transfer, no execution; never required for
    correctness."""
    import jax
    from jax.experimental.shard_map import shard_map
    from jax.sharding import Mesh, PartitionSpec

    import concourse.mybir as mybir
    from concourse import bass2jax

    nc = _get_nc()
    bass2jax.install_neuronx_cc_hook()
    partition_name = nc.partition_id_tensor.name if nc.partition_id_tensor else None
    in_specs, out_names, out_avals = [], [], []
    for alloc in nc.m.functions[0].allocations:
        if not isinstance(alloc, mybir.MemoryLocationSet):
            continue
        name = alloc.memorylocations[0].name
        shape = tuple(alloc.tensor_shape)
        dtype = mybir.dt.np(alloc.dtype)
        if alloc.kind == "ExternalInput":
            if name != partition_name:
                in_specs.append(jax.ShapeDtypeStruct(shape, dtype))
        elif alloc.kind == "ExternalOutput":
            out_names.append(name)
            out_avals.append(jax.core.ShapedArray(shape, dtype))
    n_params = len(in_specs)
    all_in_names = [
        alloc.memorylocations[0].name
        for alloc in nc.m.functions[0].allocations
        if isinstance(alloc, mybir.MemoryLocationSet)
        and alloc.kind == "ExternalInput"
        and alloc.memorylocations[0].name != partition_name
    ] + out_names
    if partition_name is not None:
        all_in_names.append(partition_name)

    def _body(*args):
        operands = list(args)
        if partition_name is not None:
            operands.append(bass2jax.partition_id_tensor())
        return tuple(
            bass2jax._bass_exec_p.bind(
                *operands,
                out_avals=tuple(out_avals),
                in_names=tuple(all_in_names),
                out_names=tuple(out_names),
                lowering_input_output_aliases=(),
                sim_require_finite=True,
                sim_require_nnan=True,
                nc=nc,
            )
        )

    devices = jax.devices()[:N_CORES]
    mesh = Mesh(np.asarray(devices), ("core",))
    n_all = n_params + len(out_names)
    donate = tuple(range(n_params, n_all))
    sharded = jax.jit(
        shard_map(
            _body,
            mesh=mesh,
            in_specs=(PartitionSpec("core"),) * n_all,
            out_specs=(PartitionSpec("core"),) * len(out_names),
            check_rep=False,
        ),
        donate_argnums=donate,
        keep_unused=True,
    )
    args = [
        jax.ShapeDtypeStruct((N_CORES * s.shape[0], *s.shape[1:]), s.dtype)
        for s in in_specs
    ] + [
        jax.ShapeDtypeStruct((N_CORES * a.shape[0], *a.shape[1:]), a.dtype)
        for a in out_avals
    ]
    sharded.lower(*args).compile()


try:
    _warm_compile()
except Exception:
    pass  # warmup is opportunistic; kernel() compiles on demand regardless


def kernel(features, hidden, W1, b1, W2, b2, V, bv):
    global LAST_EXEC_TIME_NS, LAST_RESULTS
    features = np.ascontiguousarray(np.asarray(features, dtype=np.float32))
    assert features.shape == (B, D)

    nc = _get_nc()
    in_maps = [{"x": features[i * ROWS : (i + 1) * ROWS]} for i in range(N_CORES)]
    trace = bool(os.environ.get("KERNEL_TRACE"))
    try:
        res = run_bass_kernel_spmd(
            nc, in_maps, core_ids=list(range(N_CORES)), trace=trace
        )
    except ModuleNotFoundError:
        # Tracing was requested (trace=True or BASS_TRACE) but this axon
        # client has no NTFF profile hook; rerun untraced.
        os.environ["BASS_NEVER_TRACE"] = "1"
        res = run_bass_kernel_spmd(
            nc, in_maps, core_ids=list(range(N_CORES)), trace=False
        )
    LAST_EXEC_TIME_NS = res.exec_time_ns
    LAST_RESULTS = res

    context = np.concatenate([r["ctx"] for r in res.results], axis=0)
    attn = np.concatenate(
        [r["attn"].reshape(-1) for r in res.results]
    ).reshape(B, 1, 1)
    return context, attn
